# revision 11
# baseline (speedup 1.0000x reference)
"""Trainium2 Bass kernel for nn_Attention_12197707121249 (Swin-V2-style cosine
attention with MoH gating + CPB relative position bias).

Sharding: data-parallel over batch B=8 across the 8 NeuronCores (core b owns
batch element b end-to-end; no collectives). Host-side prep is layout +
pure-function-of-inputs precompute: weight transposes, bf16 casts, the
(batch-independent) CPB bias table lookup bias[h,m,n] = exp(tbl[rpi[n,m],h]),
and the MoH gate tensor g16[b,n,h] (softmax/top-2 routing of x against the
tiny gate weights, exactly as the reference computes it).

Device pipeline per core (matmuls bf16/fp16, fp32 accumulate):
  1. qkv = x @ qkv_w.T   as [token, 3*DIM] tiles in PSUM
  2. q-side: cosine-normalize + fold softplus(temperature)*log(H*W) scale and
     query_embedding (DVE); k-side: only compute 1/|k| per token (the k
     normalization is folded into the exp activation's per-partition scale)
  3. q,k -> [d, token] layout via DMA-xbar transposes (off PE/DVE)
  4. scores^T[m,n] = kT.T @ qT per head, K=64 row-tiled so the two heads of a
     pair run concurrently on the PE array
  5. P^T = Exp((S^T)*reck[m] - 40) * expbias  (constant-shift softmax)
  6. out^T[e,n] = sum_m v65[m, e|1] @ P^T[m,n] -- appended ones column yields
     the softmax denominator in PSUM row 64 for free
  7. epilogue: 1/denom via reciprocal_approx_fast, gates*recip broadcast over
     e via tiny sel matmuls, final proj with PSUM accumulation
"""
import sys

sys.path.insert(0, "/opt/trn_rl_repo")

import numpy as np
import ml_dtypes

import concourse.bass as bass
import concourse.tile as tile
from concourse import bacc, mybir
from concourse.bass import ts
from concourse.bass_utils import run_bass_kernel_spmd

F32 = mybir.dt.float32
BF16 = mybir.dt.bfloat16
FP16 = mybir.dt.float16
AF = mybir.ActivationFunctionType
ALU = mybir.AluOpType

DIM = 1024
NH = 16
HD = 64
N = 1024
B = 8
T = 3969
NPAIR = NH // 2
TCH = 8
CCH = 8
SHIFT = -40.0

_CACHE = {}


def _bcast(ext_ap, parts, free):
    """DRAM [1, free] row -> AP that reads it into [parts, free] partitions."""
    return bass.AP(tensor=ext_ap.tensor, offset=ext_ap.offset, ap=[[0, parts], [1, free]])


def _build(use_qkvb=True, use_projb=True):
    nc = bacc.Bacc("TRN2", target_bir_lowering=False, debug=False, num_devices=B)

    xT_e = nc.dram_tensor("xT", [DIM, N], BF16, kind="ExternalInput").ap()
    qkvwT_e = nc.dram_tensor("qkvwT", [DIM, 3 * DIM], BF16, kind="ExternalInput").ap()
    qkvb_e = nc.dram_tensor("qkvb", [1, 3 * DIM], BF16, kind="ExternalInput").ap()
    projT_e = nc.dram_tensor("projT", [DIM, DIM], BF16, kind="ExternalInput").ap()
    projb_e = nc.dram_tensor("projb", [1, DIM], BF16, kind="ExternalInput").ap()
    qe_e = nc.dram_tensor("qe", [1, DIM], FP16, kind="ExternalInput").ap()
    scl_e = nc.dram_tensor("scl", [1, NH], F32, kind="ExternalInput").ap()
    biasT_e = nc.dram_tensor("biasT", [NH, N, N], BF16, kind="ExternalInput").ap()
    sel8_e = nc.dram_tensor("sel8", [8, 4 * 128], BF16, kind="ExternalInput").ap()
    g16a_e = nc.dram_tensor("g16a", [8, N], F32, kind="ExternalInput").ap()
    g16b_e = nc.dram_tensor("g16b", [8, N], F32, kind="ExternalInput").ap()
    out_e = nc.dram_tensor("out", [N, DIM], F32, kind="ExternalOutput").ap()

    from contextlib import ExitStack

    with tile.TileContext(nc) as tc, ExitStack() as stack:
        consts = stack.enter_context(tc.tile_pool(name="consts", bufs=1))
        persist = stack.enter_context(tc.tile_pool(name="persist", bufs=1))
        # bias pool opened at top level so bias prefetch can run during
        # phase 1 (no SBUF-region WAR on the phase-1 pools)
        biasp = stack.enter_context(tc.tile_pool(name="biasp", bufs=8))

        qe_b = consts.tile([128, DIM], FP16, tag="qe_b")
        nc.sync.dma_start(out=qe_b, in_=_bcast(qe_e, 128, DIM))
        scl_b = consts.tile([128, NH], F32, tag="scl_b")
        nc.sync.dma_start(out=scl_b, in_=_bcast(scl_e, 128, NH))
        sel8_sb = consts.tile([8, 4, 128], BF16, tag="sel8")
        nc.sync.dma_start(out=sel8_sb, in_=sel8_e.rearrange("p (b f) -> p b f", f=128))
        if use_qkvb:
            qkvb_sb = consts.tile([1, 3 * DIM], BF16, tag="qkvb")
            nc.sync.dma_start(out=qkvb_sb, in_=qkvb_e)
        if use_projb:
            projb_sb = consts.tile([1, DIM], BF16, tag="projb")
            nc.sync.dma_start(out=projb_sb, in_=projb_e)
        g16_sb = [consts.tile([8, N], F32, tag=f"g16_{i}", name=f"g16_{i}") for i in range(2)]
        nc.sync.dma_start(out=g16_sb[0], in_=g16a_e)
        nc.sync.dma_start(out=g16_sb[1], in_=g16b_e)
        ones1 = consts.tile([1, 128], BF16, tag="ones1")
        nc.vector.memset(ones1, 1.0)
        shiftc = consts.tile([128, 1], F32, tag="shiftc")
        nc.vector.memset(shiftc, SHIFT)

        # persistent per-core tensors
        qT = [persist.tile([128, N], FP16, tag=f"qT{a}", name=f"qT{a}") for a in range(NPAIR)]
        kT = [persist.tile([128, N], FP16, tag=f"kT{a}", name=f"kT{a}") for a in range(NPAIR)]
        v65 = [persist.tile([128, NH, 65], BF16, tag=f"v65{t}", name=f"v65{t}") for t in range(TCH)]
        outgT = [persist.tile([128, N], BF16, tag=f"og{a}", name=f"og{a}") for a in range(NPAIR)]
        reck = persist.tile([128, TCH, NH], F32, tag="reck", name="reck")
        dall = [persist.tile([8, N], BF16, tag=f"dall{i}", name=f"dall{i}") for i in range(2)]

        # ---------------- phase 1: qkv + q-norm + transposes -------------------
        with (
            tc.tile_pool(name="w", bufs=1) as wpool,
            tc.tile_pool(name="ntmp", bufs=2) as ntmp,
            tc.tile_pool(name="qkvps", bufs=8, space="PSUM") as qkvps,
        ):
            w_sb = [wpool.tile([128, 3 * DIM], BF16, tag=f"w{c}", name=f"w{c}") for c in range(CCH)]
            xT_sb = [wpool.tile([128, N], BF16, tag=f"xT{c}", name=f"xT{c}") for c in range(CCH)]
            for c in range(CCH):
                nc.gpsimd.dma_start(out=xT_sb[c], in_=xT_e[ts(c, 128), :])
            for c in range(CCH):
                nc.gpsimd.dma_start(out=w_sb[c], in_=qkvwT_e[ts(c, 128), :])

            for t in range(TCH):
                ps = [qkvps.tile([128, 512], F32, tag="qkv", bufs=8, name=f"qkvps{j}") for j in range(6)]
                for j in range(6):
                    for c in range(CCH):
                        nc.tensor.matmul(
                            ps[j], xT_sb[c][:, ts(t, 128)], w_sb[c][:, ts(j, 512)],
                            start=(c == 0), stop=(not use_qkvb and c == CCH - 1),
                        )
                    if use_qkvb:
                        nc.tensor.matmul(
                            ps[j], ones1, qkvb_sb[:, ts(j, 512)], start=False, stop=True,
                        )

                # q,k -> SBUF fp16 (k feeds DMA transpose directly; raw k)
                qsb = ntmp.tile([128, DIM], FP16, tag="qsb")
                nc.scalar.copy(out=qsb[:, 0:512], in_=ps[0])
                nc.scalar.copy(out=qsb[:, 512:1024], in_=ps[1])
                ksb = ntmp.tile([128, DIM], FP16, tag="ksb")
                nc.scalar.copy(out=ksb[:, 0:512], in_=ps[2])
                nc.scalar.copy(out=ksb[:, 512:1024], in_=ps[3])

                # v65: [128, h, 0:64] = v head h ; [:, :, 64] = 1.0
                for j in range(4, 6):
                    nc.scalar.copy(
                        out=v65[t][:, (j - 4) * 8 : (j - 4) * 8 + 8, 0:HD],
                        in_=ps[j].rearrange("p (g d) -> p g d", d=HD),
                    )
                nc.vector.memset(v65[t][:, :, 64:65], 1.0)

                # squared L2 norms per head
                sqh = ntmp.tile([128, DIM], FP16, tag="sqh")
                nc.vector.tensor_mul(sqh, qsb, qsb)
                sqk = ntmp.tile([128, DIM], FP16, tag="sqk")
                nc.vector.tensor_mul(sqk, ksb, ksb)
                ss = ntmp.tile([128, 32], F32, tag="ss")
                nc.vector.tensor_reduce(
                    ss[:, 0:16], sqh.rearrange("p (g d) -> p g d", d=HD),
                    axis=mybir.AxisListType.X, op=ALU.add,
                )
                nc.vector.tensor_reduce(
                    ss[:, 16:32], sqk.rearrange("p (g d) -> p g d", d=HD),
                    axis=mybir.AxisListType.X, op=ALU.add,
                )
                nc.vector.tensor_scalar_max(ss, ss, 1e-24)
                sr = ntmp.tile([128, 32], F32, tag="sr")
                nc.scalar.activation(out=sr, in_=ss, func=AF.Sqrt)
                rec = ntmp.tile([128, 32], F32, tag="rec")
                nc.vector.reciprocal_approx_fast(out=rec, in_=sr)
                # q scale = softplus(temp)*log(HW) / |q| ; k scale kept separate
                nc.vector.tensor_mul(rec[:, 0:16], rec[:, 0:16], scl_b)
                nc.vector.tensor_copy(out=reck[:, t, :], in_=rec[:, 16:32])

                # qn = qhat*scl + qe*scl  (fp16)
                qn = ntmp.tile([128, DIM], FP16, tag="qn")
                nc.vector.tensor_mul(
                    qn.rearrange("p (g d) -> p g d", d=HD),
                    qsb.rearrange("p (g d) -> p g d", d=HD),
                    rec[:, 0:16, None].to_broadcast([128, 16, HD]),
                )
                nc.vector.tensor_add(qn, qn, qe_b)

                # q,k -> [d, token] via DMA xbar transposes
                for a in range(NPAIR):
                    nc.sync.dma_start(
                        out=qT[a][:, ts(t, 128)], in_=qn[:, ts(a, 128)], transpose=True
                    )
                    nc.scalar.dma_start(
                        out=kT[a][:, ts(t, 128)], in_=ksb[:, ts(a, 128)], transpose=True
                    )

        # ---------------- phase 2: attention -------------------
        with (
            tc.tile_pool(name="pw", bufs=1) as pwpool,
            tc.tile_pool(name="ptp", bufs=5) as ptp,
            tc.tile_pool(name="etp", bufs=4) as etp,
            tc.tile_pool(name="pvsb", bufs=8) as pvsb,
            tc.tile_pool(name="fpp", bufs=1) as fpp,
            tc.tile_pool(name="att", bufs=8, space="PSUM") as attps,
        ):
            pw_sb = [pwpool.tile([128, DIM], BF16, tag=f"pw{c}", name=f"pw{c}") for c in range(CCH)]
            for c in range(CCH):
                nc.gpsimd.dma_start(out=pw_sb[c], in_=projT_e[ts(c, 128), :])

            psb = {}

            def emit_batch_epilogue(bb):
                # pairs 4*bb .. 4*bb+3; gates and denominators are row-aligned
                dallf = fpp.tile([8, N], F32, tag="dallf")
                nc.vector.tensor_copy(out=dallf, in_=dall[bb])
                rinv = fpp.tile([8, N], F32, tag="rinv")
                nc.vector.reciprocal_approx_fast(out=rinv, in_=dallf)
                fp = fpp.tile([8, N], BF16, tag="fp")
                nc.vector.tensor_mul(fp, g16_sb[bb], rinv)
                for bp in range(4):
                    a = 4 * bb + bp
                    for half in range(2):
                        bf = attps.tile([128, 512], F32, tag="st", bufs=4, name="bf")
                        nc.tensor.matmul(
                            bf, sel8_sb[:, bp, :], fp[:, ts(half, 512)],
                            start=True, stop=True,
                        )
                        nc.vector.tensor_mul(
                            outgT[a][0:64, ts(half, 512)],
                            psb[(a, 0)][0:64, ts(half, 512)], bf[0:64, :])
                        nc.vector.tensor_mul(
                            outgT[a][64:128, ts(half, 512)],
                            psb[(a, 1)][0:64, ts(half, 512)], bf[64:128, :])

            for a in range(NPAIR):
                pv2 = [attps.tile([65, N], F32, tag="pv", bufs=2, name=f"pv{a}_{r}") for r in range(2)]

                def emit_pv(mc_, pts_):
                    for r_ in range(2):
                        for half_ in range(2):
                            nc.tensor.matmul(
                                pv2[r_][0:65, ts(half_, 512)], v65[mc_][:, 2 * a + r_, :],
                                pts_[r_][:, ts(half_, 512)],
                                start=(mc_ == 0), stop=(mc_ == 7),
                            )

                prev = None
                for mc in range(8):
                    cur = []
                    for r in range(2):
                        h = 2 * a + r
                        bt = biasp.tile([128, N], BF16, tag="bias")
                        nc.gpsimd.dma_start(out=bt, in_=biasT_e[h, ts(mc, 128), :])
                        pt = ptp.tile([128, N], BF16, tag="pt")
                        for half in range(2):
                            st = attps.tile([128, 512], F32, tag="st", bufs=4, name="st")
                            nc.tensor.matmul(
                                st,
                                kT[a][64 * r : 64 * r + 64, ts(mc, 128)],
                                qT[a][64 * r : 64 * r + 64, ts(half, 512)],
                                start=True, stop=True,
                            )
                            et = etp.tile([128, 512], BF16, tag="et")
                            nc.scalar.activation(
                                out=et, in_=st, func=AF.Exp,
                                bias=shiftc, scale=reck[:, mc, h : h + 1],
                            )
                            nc.vector.tensor_mul(pt[:, ts(half, 512)], et, bt[:, ts(half, 512)])
                        cur.append(pt)
                    if prev is not None:
                        emit_pv(mc - 1, prev)
                    prev = cur
                emit_pv(7, prev)
                for r in range(2):
                    p = pvsb.tile([65, N], BF16, tag="psb", name=f"psb{a}_{r}")
                    nc.vector.tensor_copy(out=p, in_=pv2[r][0:65, :])
                    psb[(a, r)] = p
                    nc.sync.dma_start(
                        out=dall[a // 4][2 * (a % 4) + r : 2 * (a % 4) + r + 1, :],
                        in_=p[64:65, :],
                    )
                if a == 3:
                    emit_batch_epilogue(0)
            emit_batch_epilogue(1)

            # ---------------- phase 3: proj -------------------
            with tc.tile_pool(name="osb", bufs=3) as osb:
                for t in range(TCH):
                    ot = osb.tile([128, DIM], F32, tag="ot")
                    for o in range(2):
                        pp = attps.tile([128, 512], F32, tag="st", bufs=4, name="opp")
                        for c in range(CCH):
                            nc.tensor.matmul(
                                pp, outgT[c][:, ts(t, 128)], pw_sb[c][:, ts(o, 512)],
                                start=(c == 0), stop=(not use_projb and c == CCH - 1),
                            )
                        if use_projb:
                            nc.tensor.matmul(pp, ones1, projb_sb[:, ts(o, 512)], start=False, stop=True)
                        if o == 0:
                            nc.scalar.copy(out=ot[:, ts(o, 512)], in_=pp)
                        else:
                            nc.vector.tensor_copy(out=ot[:, ts(o, 512)], in_=pp)
                    nc.sync.dma_start(out=out_e[ts(t, 128), :], in_=ot)

    nc.compile()
    return nc


def _prep(inputs):
    x = np.asarray(inputs["x"], np.float32)
    rct = np.asarray(inputs["relative_coords_table"], np.float32)
    rpi = np.asarray(inputs["relative_pos_index"])
    H = int(np.asarray(inputs["H"])); W = int(np.asarray(inputs["W"]))
    bf = ml_dtypes.bfloat16

    tbl = np.maximum(rct @ np.asarray(inputs["cpb1_w"], np.float32).T
                     + np.asarray(inputs["cpb1_b"], np.float32), 0.0)
    tbl = tbl @ np.asarray(inputs["cpb2_w"], np.float32).T + np.asarray(inputs["cpb2_b"], np.float32)
    biasT = np.exp(np.ascontiguousarray(tbl.T[:, rpi.T])).astype(bf)  # exp(bias) [h, m, n]

    temp = np.asarray(inputs["temperature"], np.float32).reshape(NH)
    scale = np.logaddexp(0.0, temp) * np.log(float(H * W))
    qe = np.asarray(inputs["query_embedding"], np.float32).reshape(NH, HD)
    qe_row = (qe * scale[:, None]).reshape(1, NH * HD).astype(np.float16)

    # ---- MoH gating, computed exactly as the reference does ----
    _x = x.reshape(B * N, DIM)

    def _softmax(z):
        z = z - z.max(axis=1, keepdims=True)
        e = np.exp(z)
        return e / e.sum(axis=1, keepdims=True)

    wg_w = np.asarray(inputs["wg_w"], np.float32)
    wg0_w = np.asarray(inputs["wg0_w"], np.float32)
    wg1_w = np.asarray(inputs["wg1_w"], np.float32)
    gates = _softmax(_x @ wg_w.T)                       # [BN, 8]
    order = np.argsort(-gates, axis=1, kind="stable")   # top-2 like lax.top_k
    idx = order[:, :2]
    mask = np.zeros_like(gates)
    np.put_along_axis(mask, idx, 1.0, axis=1)
    routed = gates * mask
    routed = routed / np.clip(routed.sum(axis=1, keepdims=True),
                              np.finfo(np.float32).eps, None)
    routed = routed.reshape(B, N, 8) * 2.0
    shared = _softmax(_x @ wg1_w.T).reshape(B, N, 8) * 8.0
    w0 = _softmax(_x @ wg0_w.T).reshape(B, N, 2) * 2.0
    g16 = np.concatenate([shared * w0[..., :1], routed * w0[..., 1:]], axis=2)  # [B, N, 16]

    sel8 = np.zeros((8, 4, 128), np.float32)
    for bp in range(4):
        sel8[2 * bp, bp, 0:64] = 1.0
        sel8[2 * bp + 1, bp, 64:128] = 1.0

    shared_m = {
        "qkvwT": np.ascontiguousarray(np.asarray(inputs["qkv_w"], np.float32).T).astype(bf),
        "qkvb": np.asarray(inputs["qkv_b"], np.float32).reshape(1, -1).astype(bf),
        "projT": np.ascontiguousarray(np.asarray(inputs["proj_w"], np.float32).T).astype(bf),
        "projb": np.asarray(inputs["proj_b"], np.float32).reshape(1, -1).astype(bf),
        "qe": qe_row,
        "scl": scale.reshape(1, NH).astype(np.float32),
        "biasT": biasT,
        "sel8": sel8.reshape(8, 4 * 128).astype(bf),
    }
    in_maps = []
    for b in range(B):
        m = dict(shared_m)
        m["xT"] = np.ascontiguousarray(x[b].T).astype(bf)
        m["g16a"] = np.ascontiguousarray(g16[b, :, 0:8].T.astype(np.float32))
        m["g16b"] = np.ascontiguousarray(g16[b, :, 8:16].T.astype(np.float32))
        in_maps.append(m)
    return in_maps


def _execute(inputs, trace=False):
    use_qkvb = bool(np.any(np.asarray(inputs["qkv_b"])))
    use_projb = bool(np.any(np.asarray(inputs["proj_b"])))
    key = ("nc", use_qkvb, use_projb)
    if key not in _CACHE:
        _CACHE[key] = _build(use_qkvb, use_projb)
    nc = _CACHE[key]
    in_maps = _prep(inputs)
    res = run_bass_kernel_spmd(nc, in_maps, list(range(B)), trace=trace)
    out = np.stack([res.results[b]["out"] for b in range(B)], axis=0)
    return out, res


def kernel(**inputs):
    out, _ = _execute(inputs, trace=False)
    return out


# revision 19
# speedup vs baseline: 1.3380x; 1.3380x over previous
"""Trainium2 Bass kernel for nn_Attention_12197707121249 (Swin-V2-style cosine
attention with MoH gating + CPB relative position bias).

Sharding: data-parallel over batch B=8 across the 8 NeuronCores (core b owns
batch element b end-to-end; no collectives). Host-side prep is layout +
pure-function-of-inputs precompute: weight transposes, bf16 casts, the
(batch-independent) CPB bias table lookup bias[h,m,n] = exp(tbl[rpi[n,m],h]),
and the MoH gate tensor g16[b,n,h] (softmax/top-2 routing of x against the
tiny gate weights, exactly as the reference computes it).

Device pipeline per core (matmuls bf16/fp16, fp32 accumulate):
  1. qkv = x @ qkv_w.T   as [token, 3*DIM] tiles in PSUM
  2. q-side: cosine-normalize + fold softplus(temperature)*log(H*W) scale and
     query_embedding (DVE); k-side: only compute 1/|k| per token (the k
     normalization is folded into the exp activation's per-partition scale)
  3. q,k -> [d, token] layout via DMA-xbar transposes (off PE/DVE)
  4. scores^T[m,n] = kT.T @ qT per head, K=64 row-tiled so the two heads of a
     pair run concurrently on the PE array
  5. P^T = Exp((S^T)*reck[m] - 40) * expbias  (constant-shift softmax)
  6. out^T[e,n] = sum_m v65[m, e|1] @ P^T[m,n] -- appended ones column yields
     the softmax denominator in PSUM row 64 for free
  7. epilogue: 1/denom via reciprocal_approx_fast, gates*recip broadcast over
     e via tiny sel matmuls, final proj with PSUM accumulation
"""
import sys

sys.path.insert(0, "/opt/trn_rl_repo")

import numpy as np
import ml_dtypes

import concourse.bass as bass
import concourse.tile as tile
from concourse import bacc, mybir
from concourse.bass import ts
from concourse.bass_utils import run_bass_kernel_spmd
from concourse.masks import make_identity

F32 = mybir.dt.float32
BF16 = mybir.dt.bfloat16
FP16 = mybir.dt.float16
AF = mybir.ActivationFunctionType
ALU = mybir.AluOpType

DIM = 1024
NH = 16
HD = 64
N = 1024
B = 8
T = 3969
NPAIR = NH // 2
TCH = 8
CCH = 8
SHIFT = -40.0

_CACHE = {}


def _bcast(ext_ap, parts, free):
    """DRAM [1, free] row -> AP that reads it into [parts, free] partitions."""
    return bass.AP(tensor=ext_ap.tensor, offset=ext_ap.offset, ap=[[0, parts], [1, free]])


def _build(use_qkvb=True, use_projb=True):
    nc = bacc.Bacc("TRN2", target_bir_lowering=False, debug=False, num_devices=B)

    xT_e = nc.dram_tensor("xT", [DIM, N], BF16, kind="ExternalInput").ap()
    qkvwT_e = nc.dram_tensor("qkvwT", [DIM, 3 * DIM], BF16, kind="ExternalInput").ap()
    qkvb_e = nc.dram_tensor("qkvb", [1, 3 * DIM], BF16, kind="ExternalInput").ap()
    projT_e = nc.dram_tensor("projT", [DIM, DIM], BF16, kind="ExternalInput").ap()
    projb_e = nc.dram_tensor("projb", [1, DIM], BF16, kind="ExternalInput").ap()
    qe_e = nc.dram_tensor("qe", [1, DIM], FP16, kind="ExternalInput").ap()
    scl_e = nc.dram_tensor("scl", [1, NH], F32, kind="ExternalInput").ap()
    biasT_e = nc.dram_tensor("biasT", [NH, N, N], BF16, kind="ExternalInput").ap()
    sel8_e = nc.dram_tensor("sel8", [8, 4 * 128], BF16, kind="ExternalInput").ap()
    g16a_e = nc.dram_tensor("g16a", [8, N], F32, kind="ExternalInput").ap()
    g16b_e = nc.dram_tensor("g16b", [8, N], F32, kind="ExternalInput").ap()
    out_e = nc.dram_tensor("out", [N, DIM], F32, kind="ExternalOutput").ap()

    from contextlib import ExitStack

    with tile.TileContext(nc) as tc, ExitStack() as stack:
        consts = stack.enter_context(tc.tile_pool(name="consts", bufs=1))
        persist = stack.enter_context(tc.tile_pool(name="persist", bufs=1))
        # bias pool opened at top level so bias prefetch can run during
        # phase 1 (no SBUF-region WAR on the phase-1 pools)
        biasp = stack.enter_context(tc.tile_pool(name="biasp", bufs=8))

        qe_b = consts.tile([128, DIM], FP16, tag="qe_b")
        nc.sync.dma_start(out=qe_b, in_=_bcast(qe_e, 128, DIM))
        scl_b = consts.tile([128, NH], F32, tag="scl_b")
        nc.sync.dma_start(out=scl_b, in_=_bcast(scl_e, 128, NH))
        sel8_sb = consts.tile([8, 4, 128], BF16, tag="sel8")
        nc.sync.dma_start(out=sel8_sb, in_=sel8_e.rearrange("p (b f) -> p b f", f=128))
        if use_qkvb:
            qkvb_sb = consts.tile([1, 3 * DIM], BF16, tag="qkvb")
            nc.sync.dma_start(out=qkvb_sb, in_=qkvb_e)
        if use_projb:
            projb_sb = consts.tile([1, DIM], BF16, tag="projb")
            nc.sync.dma_start(out=projb_sb, in_=projb_e)
        g16_sb = [consts.tile([8, N], F32, tag=f"g16_{i}", name=f"g16_{i}") for i in range(2)]
        nc.sync.dma_start(out=g16_sb[0], in_=g16a_e)
        nc.sync.dma_start(out=g16_sb[1], in_=g16b_e)
        ones1 = consts.tile([1, 128], BF16, tag="ones1")
        nc.vector.memset(ones1, 1.0)
        shiftc = consts.tile([128, 1], F32, tag="shiftc")
        nc.vector.memset(shiftc, SHIFT)
        identh = consts.tile([128, 128], FP16, tag="identh")
        make_identity(nc, identh)

        # persistent per-core tensors
        qT = [persist.tile([128, N], FP16, tag=f"qT{a}", name=f"qT{a}") for a in range(NPAIR)]
        kT = [persist.tile([128, N], FP16, tag=f"kT{a}", name=f"kT{a}") for a in range(NPAIR)]
        v65 = [persist.tile([128, NH, 65], BF16, tag=f"v65{t}", name=f"v65{t}") for t in range(TCH)]
        outgT = [persist.tile([128, N], BF16, tag=f"og{a}", name=f"og{a}") for a in range(NPAIR)]
        dall = [persist.tile([8, N], BF16, tag=f"dall{i}", name=f"dall{i}") for i in range(2)]

        # ---------------- phase 1: qkv + q-norm + transposes -------------------
        with (
            tc.tile_pool(name="w", bufs=1) as wpool,
            tc.tile_pool(name="ntmp", bufs=2) as ntmp,
            tc.tile_pool(name="qkvps", bufs=6, space="PSUM") as qkvps,
        ):
            w_sb = [wpool.tile([128, 3 * DIM], BF16, tag=f"w{c}", name=f"w{c}") for c in range(CCH)]
            xT_sb = [wpool.tile([128, N], BF16, tag=f"xT{c}", name=f"xT{c}") for c in range(CCH)]
            for c in range(CCH):
                nc.gpsimd.dma_start(out=xT_sb[c], in_=xT_e[ts(c, 128), :])
            for c in range(CCH):
                nc.gpsimd.dma_start(out=w_sb[c], in_=qkvwT_e[ts(c, 128), :])

            for t in range(TCH):
                ps = [qkvps.tile([128, 512], F32, tag="qkv", bufs=6, name=f"qkvps{j}") for j in range(6)]
                for j in range(6):
                    for c in range(CCH):
                        nc.tensor.matmul(
                            ps[j], xT_sb[c][:, ts(t, 128)], w_sb[c][:, ts(j, 512)],
                            start=(c == 0), stop=(not use_qkvb and c == CCH - 1),
                        )
                    if use_qkvb:
                        nc.tensor.matmul(
                            ps[j], ones1, qkvb_sb[:, ts(j, 512)], start=False, stop=True,
                        )

                # q,k -> SBUF bf16 staging
                qkv_sb = ntmp.tile([128, 2 * DIM], BF16, tag="qkv_sb")
                for j in range(4):
                    nc.scalar.copy(out=qkv_sb[:, ts(j, 512)], in_=ps[j])

                # v65: [128, h, 0:64] = v head h ; [:, :, 64] = 1.0
                for j in range(4, 6):
                    nc.scalar.copy(
                        out=v65[t][:, (j - 4) * 8 : (j - 4) * 8 + 8, 0:HD],
                        in_=ps[j].rearrange("p (g d) -> p g d", d=HD),
                    )
                nc.vector.memset(v65[t][:, :, 64:65], 1.0)

                # squared L2 norms per head (q and k)
                sqh = ntmp.tile([128, 2 * DIM], FP16, tag="sqh")
                nc.vector.tensor_mul(sqh, qkv_sb, qkv_sb)
                ss = ntmp.tile([128, 32], F32, tag="ss")
                nc.vector.tensor_reduce(
                    ss, sqh.rearrange("p (g d) -> p g d", d=HD),
                    axis=mybir.AxisListType.X, op=ALU.add,
                )
                nc.vector.tensor_scalar_max(ss, ss, 1e-24)
                sr = ntmp.tile([128, 32], F32, tag="sr")
                nc.scalar.activation(out=sr, in_=ss, func=AF.Sqrt)
                rec = ntmp.tile([128, 32], F32, tag="rec")
                nc.vector.reciprocal_approx_fast(out=rec, in_=sr)
                # fold softplus(temp)*log(HW) scale into the q-side recips
                nc.vector.tensor_mul(rec[:, 0:16], rec[:, 0:16], scl_b)

                # qkn = [qhat*scl + qe*scl | khat]  (fp16)
                qkn = ntmp.tile([128, 2 * DIM], FP16, tag="qkn")
                nc.vector.tensor_mul(
                    qkn.rearrange("p (g d) -> p g d", d=HD),
                    qkv_sb.rearrange("p (g d) -> p g d", d=HD),
                    rec[:, :, None].to_broadcast([128, 32, HD]),
                )
                nc.vector.tensor_add(qkn[:, :DIM], qkn[:, :DIM], qe_b)

                # q,k -> [d, token] via PE transposes
                for a in range(NPAIR):
                    tq = qkvps.tile([128, 128], FP16, tag="tr", bufs=2, name="tq")
                    nc.tensor.transpose(tq, qkn[:, ts(a, 128)], identh)
                    nc.vector.tensor_copy(out=qT[a][:, ts(t, 128)], in_=tq)
                    tk = qkvps.tile([128, 128], FP16, tag="tr", bufs=2, name="tk")
                    nc.tensor.transpose(tk, qkn[:, DIM + a * 128 : DIM + a * 128 + 128], identh)
                    nc.vector.tensor_copy(out=kT[a][:, ts(t, 128)], in_=tk)

        # ---------------- phase 2: attention -------------------
        with (
            tc.tile_pool(name="pw", bufs=1) as pwpool,
            tc.tile_pool(name="ptp", bufs=5) as ptp,
            tc.tile_pool(name="etp", bufs=4) as etp,
            tc.tile_pool(name="pvsb", bufs=8) as pvsb,
            tc.tile_pool(name="fpp", bufs=1) as fpp,
            tc.tile_pool(name="att", bufs=8, space="PSUM") as attps,
        ):
            pw_sb = [pwpool.tile([128, DIM], BF16, tag=f"pw{c}", name=f"pw{c}") for c in range(CCH)]
            for c in range(CCH):
                nc.gpsimd.dma_start(out=pw_sb[c], in_=projT_e[ts(c, 128), :])

            psb = {}

            def emit_batch_epilogue(bb):
                # pairs 4*bb .. 4*bb+3; gates and denominators are row-aligned
                dallf = fpp.tile([8, N], F32, tag="dallf")
                nc.vector.tensor_copy(out=dallf, in_=dall[bb])
                rinv = fpp.tile([8, N], F32, tag="rinv")
                nc.vector.reciprocal_approx_fast(out=rinv, in_=dallf)
                fp = fpp.tile([8, N], BF16, tag="fp")
                nc.vector.tensor_mul(fp, g16_sb[bb], rinv)
                for bp in range(4):
                    a = 4 * bb + bp
                    for half in range(2):
                        bf = attps.tile([128, 512], F32, tag="st", bufs=2, name="bf")
                        nc.tensor.matmul(
                            bf, sel8_sb[:, bp, :], fp[:, ts(half, 512)],
                            start=True, stop=True,
                        )
                        nc.vector.tensor_mul(
                            outgT[a][0:64, ts(half, 512)],
                            psb[(a, 0)][0:64, ts(half, 512)], bf[0:64, :])
                        nc.vector.tensor_mul(
                            outgT[a][64:128, ts(half, 512)],
                            psb[(a, 1)][0:64, ts(half, 512)], bf[64:128, :])

            for a in range(NPAIR):
                pv2 = [attps.tile([65, N], F32, tag="pv", bufs=2, name=f"pv{a}_{r}") for r in range(2)]

                def emit_pv(mc_, pts_):
                    for r_ in range(2):
                        for half_ in range(2):
                            nc.tensor.matmul(
                                pv2[r_][0:65, ts(half_, 512)], v65[mc_][:, 2 * a + r_, :],
                                pts_[r_][:, ts(half_, 512)],
                                start=(mc_ == 0), stop=(mc_ == 7),
                            )

                prev = None
                for mc in range(8):
                    bts = []
                    sts = []
                    for r in range(2):
                        h = 2 * a + r
                        bt = biasp.tile([128, N], BF16, tag="bias")
                        nc.gpsimd.dma_start(out=bt, in_=biasT_e[h, ts(mc, 128), :])
                        bts.append(bt)
                        sts.append(attps.tile([128, N], F32, tag="st", bufs=2, name="st"))
                    # K=64 row-tiled: the two heads of the pair target disjoint
                    # PE row groups, so adjacent-issued matmuls run concurrently
                    for half in range(2):
                        for r in range(2):
                            nc.tensor.matmul(
                                sts[r][:, ts(half, 512)],
                                kT[a][64 * r : 64 * r + 64, ts(mc, 128)],
                                qT[a][64 * r : 64 * r + 64, ts(half, 512)],
                                start=True, stop=True,
                            )
                    cur = []
                    for r in range(2):
                        et = etp.tile([128, N], BF16, tag="et")
                        nc.scalar.activation(out=et, in_=sts[r], func=AF.Exp, bias=shiftc)
                        pt = ptp.tile([128, N], BF16, tag="pt")
                        nc.vector.tensor_mul(pt, et, bts[r])
                        cur.append(pt)
                    if prev is not None:
                        emit_pv(mc - 1, prev)
                    prev = cur
                emit_pv(7, prev)
                for r in range(2):
                    p = pvsb.tile([65, N], BF16, tag="psb", name=f"psb{a}_{r}")
                    nc.vector.tensor_copy(out=p, in_=pv2[r][0:65, :])
                    psb[(a, r)] = p
                    nc.sync.dma_start(
                        out=dall[a // 4][2 * (a % 4) + r : 2 * (a % 4) + r + 1, :],
                        in_=p[64:65, :],
                    )
                if a == 3:
                    emit_batch_epilogue(0)
            emit_batch_epilogue(1)

            # ---------------- phase 3: proj -------------------
            with tc.tile_pool(name="osb", bufs=3) as osb:
                for t in range(TCH):
                    ot = osb.tile([128, DIM], F32, tag="ot")
                    for o in range(2):
                        pp = attps.tile([128, 512], F32, tag="st", bufs=2, name="opp")
                        for c in range(CCH):
                            nc.tensor.matmul(
                                pp, outgT[c][:, ts(t, 128)], pw_sb[c][:, ts(o, 512)],
                                start=(c == 0), stop=(not use_projb and c == CCH - 1),
                            )
                        if use_projb:
                            nc.tensor.matmul(pp, ones1, projb_sb[:, ts(o, 512)], start=False, stop=True)
                        if o == 0:
                            nc.scalar.copy(out=ot[:, ts(o, 512)], in_=pp)
                        else:
                            nc.vector.tensor_copy(out=ot[:, ts(o, 512)], in_=pp)
                    nc.sync.dma_start(out=out_e[ts(t, 128), :], in_=ot)

    nc.compile()
    return nc


def _prep(inputs):
    x = np.asarray(inputs["x"], np.float32)
    rct = np.asarray(inputs["relative_coords_table"], np.float32)
    rpi = np.asarray(inputs["relative_pos_index"])
    H = int(np.asarray(inputs["H"])); W = int(np.asarray(inputs["W"]))
    bf = ml_dtypes.bfloat16

    tbl = np.maximum(rct @ np.asarray(inputs["cpb1_w"], np.float32).T
                     + np.asarray(inputs["cpb1_b"], np.float32), 0.0)
    tbl = tbl @ np.asarray(inputs["cpb2_w"], np.float32).T + np.asarray(inputs["cpb2_b"], np.float32)
    biasT = np.exp(np.ascontiguousarray(tbl.T[:, rpi.T])).astype(bf)  # exp(bias) [h, m, n]

    temp = np.asarray(inputs["temperature"], np.float32).reshape(NH)
    scale = np.logaddexp(0.0, temp) * np.log(float(H * W))
    qe = np.asarray(inputs["query_embedding"], np.float32).reshape(NH, HD)
    qe_row = (qe * scale[:, None]).reshape(1, NH * HD).astype(np.float16)

    # ---- MoH gating, computed exactly as the reference does ----
    _x = x.reshape(B * N, DIM)

    def _softmax(z):
        z = z - z.max(axis=1, keepdims=True)
        e = np.exp(z)
        return e / e.sum(axis=1, keepdims=True)

    wg_w = np.asarray(inputs["wg_w"], np.float32)
    wg0_w = np.asarray(inputs["wg0_w"], np.float32)
    wg1_w = np.asarray(inputs["wg1_w"], np.float32)
    gates = _softmax(_x @ wg_w.T)                       # [BN, 8]
    order = np.argsort(-gates, axis=1, kind="stable")   # top-2 like lax.top_k
    idx = order[:, :2]
    mask = np.zeros_like(gates)
    np.put_along_axis(mask, idx, 1.0, axis=1)
    routed = gates * mask
    routed = routed / np.clip(routed.sum(axis=1, keepdims=True),
                              np.finfo(np.float32).eps, None)
    routed = routed.reshape(B, N, 8) * 2.0
    shared = _softmax(_x @ wg1_w.T).reshape(B, N, 8) * 8.0
    w0 = _softmax(_x @ wg0_w.T).reshape(B, N, 2) * 2.0
    g16 = np.concatenate([shared * w0[..., :1], routed * w0[..., 1:]], axis=2)  # [B, N, 16]

    sel8 = np.zeros((8, 4, 128), np.float32)
    for bp in range(4):
        sel8[2 * bp, bp, 0:64] = 1.0
        sel8[2 * bp + 1, bp, 64:128] = 1.0

    shared_m = {
        "qkvwT": np.ascontiguousarray(np.asarray(inputs["qkv_w"], np.float32).T).astype(bf),
        "qkvb": np.asarray(inputs["qkv_b"], np.float32).reshape(1, -1).astype(bf),
        "projT": np.ascontiguousarray(np.asarray(inputs["proj_w"], np.float32).T).astype(bf),
        "projb": np.asarray(inputs["proj_b"], np.float32).reshape(1, -1).astype(bf),
        "qe": qe_row,
        "scl": scale.reshape(1, NH).astype(np.float32),
        "biasT": biasT,
        "sel8": sel8.reshape(8, 4 * 128).astype(bf),
    }
    in_maps = []
    for b in range(B):
        m = dict(shared_m)
        m["xT"] = np.ascontiguousarray(x[b].T).astype(bf)
        m["g16a"] = np.ascontiguousarray(g16[b, :, 0:8].T.astype(np.float32))
        m["g16b"] = np.ascontiguousarray(g16[b, :, 8:16].T.astype(np.float32))
        in_maps.append(m)
    return in_maps


def _execute(inputs, trace=False):
    use_qkvb = bool(np.any(np.asarray(inputs["qkv_b"])))
    use_projb = bool(np.any(np.asarray(inputs["proj_b"])))
    key = ("nc", use_qkvb, use_projb)
    if key not in _CACHE:
        _CACHE[key] = _build(use_qkvb, use_projb)
    nc = _CACHE[key]
    in_maps = _prep(inputs)
    res = run_bass_kernel_spmd(nc, in_maps, list(range(B)), trace=trace)
    out = np.stack([res.results[b]["out"] for b in range(B)], axis=0)
    return out, res


def kernel(**inputs):
    out, _ = _execute(inputs, trace=False)
    return out


# revision 24
# speedup vs baseline: 1.5266x; 1.1409x over previous
"""Trainium2 Bass kernel for nn_Attention_12197707121249 (Swin-V2-style cosine
attention with MoH gating + CPB relative position bias).

Sharding: data-parallel over batch B=8 across the 8 NeuronCores (core b owns
batch element b end-to-end; no collectives). Host-side prep is layout +
pure-function-of-inputs precompute: weight transposes, bf16 casts, the
(batch-independent) CPB bias table lookup bias[h,m,n] = exp(tbl[rpi[n,m],h]),
and the MoH gate tensor g16[b,n,h] (softmax/top-2 routing of x against the
tiny gate weights, exactly as the reference computes it).

Device pipeline per core (matmuls bf16/fp16, fp32 accumulate):
  1. qkv = x @ qkv_w.T   as [token, 3*DIM] tiles in PSUM
  2. q-side: cosine-normalize + fold softplus(temperature)*log(H*W) scale and
     query_embedding (DVE); k-side: only compute 1/|k| per token (the k
     normalization is folded into the exp activation's per-partition scale)
  3. q,k -> [d, token] layout via DMA-xbar transposes (off PE/DVE)
  4. scores^T[m,n] = kT.T @ qT per head, K=64 row-tiled so the two heads of a
     pair run concurrently on the PE array
  5. P^T = Exp((S^T)*reck[m] - 40) * expbias  (constant-shift softmax)
  6. out^T[e,n] = sum_m v65[m, e|1] @ P^T[m,n] -- appended ones column yields
     the softmax denominator in PSUM row 64 for free
  7. epilogue: 1/denom via reciprocal_approx_fast, gates*recip broadcast over
     e via tiny sel matmuls, final proj with PSUM accumulation
"""
import sys

sys.path.insert(0, "/opt/trn_rl_repo")

import numpy as np
import ml_dtypes

import concourse.bass as bass
import concourse.tile as tile
from concourse import bacc, mybir
from concourse.bass import ts
from concourse.bass_utils import run_bass_kernel_spmd
from concourse.masks import make_identity

F32 = mybir.dt.float32
BF16 = mybir.dt.bfloat16
FP16 = mybir.dt.float16
AF = mybir.ActivationFunctionType
ALU = mybir.AluOpType

DIM = 1024
NH = 16
HD = 64
N = 1024
B = 8
T = 3969
NPAIR = NH // 2
TCH = 8
CCH = 8
SHIFT = -40.0

_CACHE = {}


def _bcast(ext_ap, parts, free):
    """DRAM [1, free] row -> AP that reads it into [parts, free] partitions."""
    return bass.AP(tensor=ext_ap.tensor, offset=ext_ap.offset, ap=[[0, parts], [1, free]])


def _build(use_qkvb=True, use_projb=True):
    nc = bacc.Bacc("TRN2", target_bir_lowering=False, debug=False, num_devices=B)

    xT_e = nc.dram_tensor("xT", [DIM, N], BF16, kind="ExternalInput").ap()
    qkvwT_e = nc.dram_tensor("qkvwT", [DIM, 3 * DIM], BF16, kind="ExternalInput").ap()
    qkvb_e = nc.dram_tensor("qkvb", [1, 3 * DIM], BF16, kind="ExternalInput").ap()
    projT_e = nc.dram_tensor("projT", [DIM, DIM], BF16, kind="ExternalInput").ap()
    projb_e = nc.dram_tensor("projb", [1, DIM], BF16, kind="ExternalInput").ap()
    qe_e = nc.dram_tensor("qe", [1, DIM], FP16, kind="ExternalInput").ap()
    scl_e = nc.dram_tensor("scl", [1, NH], F32, kind="ExternalInput").ap()
    biasT_e = nc.dram_tensor("biasT", [NH, N, N], BF16, kind="ExternalInput").ap()
    sel8_e = nc.dram_tensor("sel8", [8, 4 * 128], BF16, kind="ExternalInput").ap()
    g16a_e = nc.dram_tensor("g16a", [8, N], F32, kind="ExternalInput").ap()
    g16b_e = nc.dram_tensor("g16b", [8, N], F32, kind="ExternalInput").ap()
    out_e = nc.dram_tensor("out", [N, DIM], F32, kind="ExternalOutput").ap()

    from contextlib import ExitStack

    with tile.TileContext(nc) as tc, ExitStack() as stack:
        consts = stack.enter_context(tc.tile_pool(name="consts", bufs=1))
        persist = stack.enter_context(tc.tile_pool(name="persist", bufs=1))
        # bias pool opened at top level so bias prefetch can run during
        # phase 1 (no SBUF-region WAR on the phase-1 pools)
        biasp = stack.enter_context(tc.tile_pool(name="biasp", bufs=8))

        qe_b = consts.tile([128, DIM], FP16, tag="qe_b")
        nc.sync.dma_start(out=qe_b, in_=_bcast(qe_e, 128, DIM))
        scl_b = consts.tile([128, NH], F32, tag="scl_b")
        nc.sync.dma_start(out=scl_b, in_=_bcast(scl_e, 128, NH))
        sel8_sb = consts.tile([8, 4, 128], BF16, tag="sel8")
        nc.sync.dma_start(out=sel8_sb, in_=sel8_e.rearrange("p (b f) -> p b f", f=128))
        if use_qkvb:
            qkvb_sb = consts.tile([1, 3 * DIM], BF16, tag="qkvb")
            nc.sync.dma_start(out=qkvb_sb, in_=qkvb_e)
        if use_projb:
            projb_sb = consts.tile([1, DIM], BF16, tag="projb")
            nc.sync.dma_start(out=projb_sb, in_=projb_e)
        g16_sb = [consts.tile([8, N], F32, tag=f"g16_{i}", name=f"g16_{i}") for i in range(2)]
        nc.sync.dma_start(out=g16_sb[0], in_=g16a_e)
        nc.sync.dma_start(out=g16_sb[1], in_=g16b_e)
        ones1 = consts.tile([1, 128], BF16, tag="ones1")
        nc.vector.memset(ones1, 1.0)
        shiftc = consts.tile([128, 1], F32, tag="shiftc")
        nc.vector.memset(shiftc, SHIFT)
        identh = consts.tile([128, 128], FP16, tag="identh")
        make_identity(nc, identh)

        # persistent per-core tensors.  qtz[h]: zero-padded per-head q in
        # [d, token] layout -- rows 64r..64r+64 hold head h's qn, the other 64
        # rows are zero so a full-K=128 score matmul annihilates the other
        # head of the kT pair (keeps the PE array fully active).
        qtz = [persist.tile([128, N], FP16, tag=f"qz{h}", name=f"qz{h}") for h in range(NH)]
        kT = [persist.tile([128, N], FP16, tag=f"kT{a}", name=f"kT{a}") for a in range(NPAIR)]
        v65 = [persist.tile([128, NH, 65], BF16, tag=f"v65{t}", name=f"v65{t}") for t in range(TCH)]
        outgT = [persist.tile([128, N], BF16, tag=f"og{a}", name=f"og{a}") for a in range(NPAIR)]
        dall = [persist.tile([8, N], BF16, tag=f"dall{i}", name=f"dall{i}") for i in range(2)]
        for h in range(NH):
            nc.vector.memset(qtz[h], 0.0)

        # ---------------- phase 1: qkv + q-norm + transposes -------------------
        with (
            tc.tile_pool(name="w", bufs=1) as wpool,
            tc.tile_pool(name="ntmp", bufs=2) as ntmp,
            tc.tile_pool(name="qkvps", bufs=6, space="PSUM") as qkvps,
        ):
            w_sb = [wpool.tile([128, 3 * DIM], BF16, tag=f"w{c}", name=f"w{c}") for c in range(CCH)]
            xT_sb = [wpool.tile([128, N], BF16, tag=f"xT{c}", name=f"xT{c}") for c in range(CCH)]
            for c in range(CCH):
                nc.gpsimd.dma_start(out=xT_sb[c], in_=xT_e[ts(c, 128), :])
                nc.gpsimd.dma_start(out=w_sb[c], in_=qkvwT_e[ts(c, 128), :])

            for t in range(TCH):
                ps = [qkvps.tile([128, 512], F32, tag="qkv", bufs=6, name=f"qkvps{j}") for j in range(6)]
                for j in range(6):
                    for c in range(CCH):
                        nc.tensor.matmul(
                            ps[j], xT_sb[c][:, ts(t, 128)], w_sb[c][:, ts(j, 512)],
                            start=(c == 0), stop=(not use_qkvb and c == CCH - 1),
                        )
                    if use_qkvb:
                        nc.tensor.matmul(
                            ps[j], ones1, qkvb_sb[:, ts(j, 512)], start=False, stop=True,
                        )

                # q,k -> SBUF bf16 staging
                qkv_sb = ntmp.tile([128, 2 * DIM], BF16, tag="qkv_sb")
                for j in range(4):
                    nc.scalar.copy(out=qkv_sb[:, ts(j, 512)], in_=ps[j])

                # v65: [128, h, 0:64] = v head h ; [:, :, 64] = 1.0
                for j in range(4, 6):
                    nc.scalar.copy(
                        out=v65[t][:, (j - 4) * 8 : (j - 4) * 8 + 8, 0:HD],
                        in_=ps[j].rearrange("p (g d) -> p g d", d=HD),
                    )
                nc.vector.memset(v65[t][:, :, 64:65], 1.0)

                # squared L2 norms per head (q and k); squares on ACT
                sqh = ntmp.tile([128, 2 * DIM], FP16, tag="sqh")
                nc.scalar.activation(out=sqh[:, 0:DIM], in_=qkv_sb[:, 0:DIM], func=AF.Square)
                nc.scalar.activation(out=sqh[:, DIM:], in_=qkv_sb[:, DIM:], func=AF.Square)
                ss = ntmp.tile([128, 32], F32, tag="ss")
                nc.vector.tensor_reduce(
                    ss, sqh.rearrange("p (g d) -> p g d", d=HD),
                    axis=mybir.AxisListType.X, op=ALU.add,
                )
                nc.vector.tensor_scalar_max(ss, ss, 1e-24)
                sr = ntmp.tile([128, 32], F32, tag="sr")
                nc.scalar.activation(out=sr, in_=ss, func=AF.Sqrt)
                rec = ntmp.tile([128, 32], F32, tag="rec")
                nc.vector.reciprocal_approx_fast(out=rec, in_=sr)
                # fold softplus(temp)*log(HW) scale into the q-side recips
                nc.vector.tensor_mul(rec[:, 0:16], rec[:, 0:16], scl_b)

                # qkn = [qhat*scl + qe*scl | khat]  (fp16)
                qkn = ntmp.tile([128, 2 * DIM], FP16, tag="qkn")
                nc.vector.tensor_mul(
                    qkn.rearrange("p (g d) -> p g d", d=HD),
                    qkv_sb.rearrange("p (g d) -> p g d", d=HD),
                    rec[:, :, None].to_broadcast([128, 32, HD]),
                )
                nc.vector.tensor_add(qkn[:, :DIM], qkn[:, :DIM], qe_b)

                # q,k -> [d, token] via PE transposes; q lands zero-padded
                # per-head (other head's rows stay zero from the memset)
                for a in range(NPAIR):
                    tq = qkvps.tile([128, 128], FP16, tag="tr", bufs=2, name="tq")
                    nc.tensor.transpose(tq, qkn[:, ts(a, 128)], identh)
                    for r in range(2):
                        nc.vector.tensor_copy(
                            out=qtz[2 * a + r][64 * r : 64 * r + 64, ts(t, 128)],
                            in_=tq[64 * r : 64 * r + 64, :],
                        )
                    tk = qkvps.tile([128, 128], FP16, tag="tr", bufs=2, name="tk")
                    nc.tensor.transpose(tk, qkn[:, DIM + a * 128 : DIM + a * 128 + 128], identh)
                    nc.scalar.copy(out=kT[a][:, ts(t, 128)], in_=tk)

        # ---------------- phase 2: attention -------------------
        with (
            tc.tile_pool(name="pw", bufs=1) as pwpool,
            tc.tile_pool(name="ptp", bufs=5) as ptp,
            tc.tile_pool(name="etp", bufs=4) as etp,
            tc.tile_pool(name="pvsb", bufs=8) as pvsb,
            tc.tile_pool(name="fpp", bufs=1) as fpp,
            tc.tile_pool(name="att", bufs=8, space="PSUM") as attps,
        ):
            pw_sb = [pwpool.tile([128, DIM], BF16, tag=f"pw{c}", name=f"pw{c}") for c in range(CCH)]
            for c in range(CCH):
                nc.gpsimd.dma_start(out=pw_sb[c], in_=projT_e[ts(c, 128), :])

            psb = {}

            def emit_batch_epilogue(bb):
                # pairs 4*bb .. 4*bb+3; gates and denominators are row-aligned
                dallf = fpp.tile([8, N], F32, tag="dallf")
                nc.vector.tensor_copy(out=dallf, in_=dall[bb])
                rinv = fpp.tile([8, N], F32, tag="rinv")
                nc.vector.reciprocal_approx_fast(out=rinv, in_=dallf)
                fp = fpp.tile([8, N], BF16, tag="fp")
                nc.vector.tensor_mul(fp, g16_sb[bb], rinv)
                for bp in range(4):
                    a = 4 * bb + bp
                    for half in range(2):
                        bf = attps.tile([128, 512], F32, tag="st", bufs=2, name="bf")
                        nc.tensor.matmul(
                            bf, sel8_sb[:, bp, :], fp[:, ts(half, 512)],
                            start=True, stop=True,
                        )
                        nc.vector.tensor_mul(
                            outgT[a][0:64, ts(half, 512)],
                            psb[(a, 0)][0:64, ts(half, 512)], bf[0:64, :])
                        nc.vector.tensor_mul(
                            outgT[a][64:128, ts(half, 512)],
                            psb[(a, 1)][0:64, ts(half, 512)], bf[64:128, :])

            for a in range(NPAIR):
                pv2 = [attps.tile([65, N], F32, tag="pv", bufs=2, name=f"pv{a}_{r}") for r in range(2)]

                def emit_pv(mc_, pts_):
                    for r_ in range(2):
                        for half_ in range(2):
                            nc.tensor.matmul(
                                pv2[r_][0:65, ts(half_, 512)], v65[mc_][:, 2 * a + r_, :],
                                pts_[r_][:, ts(half_, 512)],
                                start=(mc_ == 0), stop=(mc_ == 7),
                            )

                prev = None
                for mc in range(8):
                    cur = []
                    for r in range(2):
                        h = 2 * a + r
                        bt = biasp.tile([128, N], BF16, tag="bias")
                        nc.gpsimd.dma_start(out=bt, in_=biasT_e[h, ts(mc, 128), :])
                        st = attps.tile([128, N], F32, tag="st", bufs=2, name="st")
                        for half in range(2):
                            nc.tensor.matmul(
                                st[:, ts(half, 512)],
                                kT[a][:, ts(mc, 128)],
                                qtz[h][:, ts(half, 512)],
                                start=True, stop=True,
                            )
                        et = etp.tile([128, N], BF16, tag="et")
                        nc.scalar.activation(out=et, in_=st, func=AF.Exp, bias=shiftc)
                        pt = ptp.tile([128, N], BF16, tag="pt")
                        nc.vector.tensor_mul(pt, et, bt)
                        cur.append(pt)
                    if prev is not None:
                        emit_pv(mc - 1, prev)
                    prev = cur
                emit_pv(7, prev)
                for r in range(2):
                    p = pvsb.tile([65, N], BF16, tag="psb", name=f"psb{a}_{r}")
                    nc.vector.tensor_copy(out=p, in_=pv2[r][0:65, :])
                    psb[(a, r)] = p
                    nc.sync.dma_start(
                        out=dall[a // 4][2 * (a % 4) + r : 2 * (a % 4) + r + 1, :],
                        in_=p[64:65, :],
                    )
                if a == 4:
                    # batch-0 epilogue emitted one pair late so its chain
                    # fills scheduler gaps instead of starving pair 4
                    emit_batch_epilogue(0)
            emit_batch_epilogue(1)

            # ---------------- phase 3: proj -------------------
            with tc.tile_pool(name="osb", bufs=3) as osb:
                for t in range(TCH):
                    ot = osb.tile([128, DIM], F32, tag="ot")
                    for o in range(2):
                        pp = attps.tile([128, 512], F32, tag="st", bufs=2, name="opp")
                        for c in range(CCH):
                            nc.tensor.matmul(
                                pp, outgT[c][:, ts(t, 128)], pw_sb[c][:, ts(o, 512)],
                                start=(c == 0), stop=(not use_projb and c == CCH - 1),
                            )
                        if use_projb:
                            nc.tensor.matmul(pp, ones1, projb_sb[:, ts(o, 512)], start=False, stop=True)
                        if o == 0:
                            nc.scalar.copy(out=ot[:, ts(o, 512)], in_=pp)
                        else:
                            nc.vector.tensor_copy(out=ot[:, ts(o, 512)], in_=pp)
                    nc.sync.dma_start(out=out_e[ts(t, 128), :], in_=ot)

    nc.compile()
    return nc


def _prep(inputs):
    x = np.asarray(inputs["x"], np.float32)
    rct = np.asarray(inputs["relative_coords_table"], np.float32)
    rpi = np.asarray(inputs["relative_pos_index"])
    H = int(np.asarray(inputs["H"])); W = int(np.asarray(inputs["W"]))
    bf = ml_dtypes.bfloat16

    tbl = np.maximum(rct @ np.asarray(inputs["cpb1_w"], np.float32).T
                     + np.asarray(inputs["cpb1_b"], np.float32), 0.0)
    tbl = tbl @ np.asarray(inputs["cpb2_w"], np.float32).T + np.asarray(inputs["cpb2_b"], np.float32)
    biasT = np.exp(np.ascontiguousarray(tbl.T[:, rpi.T])).astype(bf)  # exp(bias) [h, m, n]

    temp = np.asarray(inputs["temperature"], np.float32).reshape(NH)
    scale = np.logaddexp(0.0, temp) * np.log(float(H * W))
    qe = np.asarray(inputs["query_embedding"], np.float32).reshape(NH, HD)
    qe_row = (qe * scale[:, None]).reshape(1, NH * HD).astype(np.float16)

    # ---- MoH gating, computed exactly as the reference does ----
    _x = x.reshape(B * N, DIM)

    def _softmax(z):
        z = z - z.max(axis=1, keepdims=True)
        e = np.exp(z)
        return e / e.sum(axis=1, keepdims=True)

    wg_w = np.asarray(inputs["wg_w"], np.float32)
    wg0_w = np.asarray(inputs["wg0_w"], np.float32)
    wg1_w = np.asarray(inputs["wg1_w"], np.float32)
    gates = _softmax(_x @ wg_w.T)                       # [BN, 8]
    order = np.argsort(-gates, axis=1, kind="stable")   # top-2 like lax.top_k
    idx = order[:, :2]
    mask = np.zeros_like(gates)
    np.put_along_axis(mask, idx, 1.0, axis=1)
    routed = gates * mask
    routed = routed / np.clip(routed.sum(axis=1, keepdims=True),
                              np.finfo(np.float32).eps, None)
    routed = routed.reshape(B, N, 8) * 2.0
    shared = _softmax(_x @ wg1_w.T).reshape(B, N, 8) * 8.0
    w0 = _softmax(_x @ wg0_w.T).reshape(B, N, 2) * 2.0
    g16 = np.concatenate([shared * w0[..., :1], routed * w0[..., 1:]], axis=2)  # [B, N, 16]

    sel8 = np.zeros((8, 4, 128), np.float32)
    for bp in range(4):
        sel8[2 * bp, bp, 0:64] = 1.0
        sel8[2 * bp + 1, bp, 64:128] = 1.0

    shared_m = {
        "qkvwT": np.ascontiguousarray(np.asarray(inputs["qkv_w"], np.float32).T).astype(bf),
        "qkvb": np.asarray(inputs["qkv_b"], np.float32).reshape(1, -1).astype(bf),
        "projT": np.ascontiguousarray(np.asarray(inputs["proj_w"], np.float32).T).astype(bf),
        "projb": np.asarray(inputs["proj_b"], np.float32).reshape(1, -1).astype(bf),
        "qe": qe_row,
        "scl": scale.reshape(1, NH).astype(np.float32),
        "biasT": biasT,
        "sel8": sel8.reshape(8, 4 * 128).astype(bf),
    }
    in_maps = []
    for b in range(B):
        m = dict(shared_m)
        m["xT"] = np.ascontiguousarray(x[b].T).astype(bf)
        m["g16a"] = np.ascontiguousarray(g16[b, :, 0:8].T.astype(np.float32))
        m["g16b"] = np.ascontiguousarray(g16[b, :, 8:16].T.astype(np.float32))
        in_maps.append(m)
    return in_maps


def _execute(inputs, trace=False):
    use_qkvb = bool(np.any(np.asarray(inputs["qkv_b"])))
    use_projb = bool(np.any(np.asarray(inputs["proj_b"])))
    key = ("nc", use_qkvb, use_projb)
    if key not in _CACHE:
        _CACHE[key] = _build(use_qkvb, use_projb)
    nc = _CACHE[key]
    in_maps = _prep(inputs)
    res = run_bass_kernel_spmd(nc, in_maps, list(range(B)), trace=trace)
    out = np.stack([res.results[b]["out"] for b in range(B)], axis=0)
    return out, res


def kernel(**inputs):
    out, _ = _execute(inputs, trace=False)
    return out
